# revision 6
# baseline (speedup 1.0000x reference)
"""Trainium2 Bass kernel for nn_Encoder (dense transformer block + attention heatmap).

Sharding: rows of (B,S) split across 8 cores (512 query rows each), zero collectives.
Each core recomputes K/V for its full batch from a per-core *rolled* copy of x so
the SPMD program is identical across cores (core's own query rows always sit at
rolled positions 0..511). Scores are computed transposed [t, s] so the softmax
denominator comes from a ones-row appended to V, and P@V needs no on-chip
transpose. The heatmap is written transposed+rolled per core and fixed on host.
"""
import sys
sys.path.insert(0, "/opt/trn_rl_repo")

import numpy as np
import ml_dtypes

import concourse.bass as bass
from concourse import bacc
import concourse.mybir as mybir
import concourse.tile as tile
from concourse.bass_utils import run_bass_kernel_spmd

F32 = mybir.dt.float32
BF16 = mybir.dt.bfloat16
BF = ml_dtypes.bfloat16

H, QD, VD = 8, 64, 64
D, DH = 512, 2048
S = 2048          # full sequence (one batch) per core
R = 512           # query rows per core
TC = S // 128     # 16 t-chunks
N_CORES = 8
LN_EPS = 1e-5

_NC_CACHE = {}


def build_nc(trace_label=None):
    nc = bacc.Bacc(None)
    # ---- per-core inputs ----
    xb = nc.dram_tensor("xb", [S, D], BF16, kind="ExternalInput")      # rolled batch (bf16)
    xq = nc.dram_tensor("xq", [R, D], F32, kind="ExternalInput")       # this core's rows (f32)
    maskin = nc.dram_tensor("maskin", [S, R], BF16, kind="ExternalInput")  # rolled 0/1 causal mask
    # ---- shared weights (host pre-cast to bf16) ----
    Wq = nc.dram_tensor("Wq", [D, D], BF16, kind="ExternalInput")
    Wk = nc.dram_tensor("Wk", [D, D], BF16, kind="ExternalInput")
    Wv = nc.dram_tensor("Wv", [D, D], BF16, kind="ExternalInput")
    Wo = nc.dram_tensor("Wo", [D, D], BF16, kind="ExternalInput")
    W1 = nc.dram_tensor("W1", [D, DH], BF16, kind="ExternalInput")
    W2 = nc.dram_tensor("W2", [DH, D], BF16, kind="ExternalInput")
    bq = nc.dram_tensor("bq", [D], F32, kind="ExternalInput")
    bk = nc.dram_tensor("bk", [D], F32, kind="ExternalInput")
    bv = nc.dram_tensor("bv", [D], F32, kind="ExternalInput")
    bo = nc.dram_tensor("bo", [D], F32, kind="ExternalInput")
    b1 = nc.dram_tensor("b1", [DH], F32, kind="ExternalInput")
    b2 = nc.dram_tensor("b2", [D], F32, kind="ExternalInput")
    g1 = nc.dram_tensor("g1", [D], F32, kind="ExternalInput")
    bt1 = nc.dram_tensor("bt1", [D], F32, kind="ExternalInput")
    g2 = nc.dram_tensor("g2", [D], F32, kind="ExternalInput")
    bt2 = nc.dram_tensor("bt2", [D], F32, kind="ExternalInput")
    identin = nc.dram_tensor("identin", [128, 128], BF16, kind="ExternalInput")
    # ---- outputs ----
    heatT = nc.dram_tensor("heatT", [H, S, R], F32, kind="ExternalOutput")
    x2out = nc.dram_tensor("x2out", [R, D], F32, kind="ExternalOutput")

    EXPF = mybir.ActivationFunctionType.Exp
    RELUF = mybir.ActivationFunctionType.Relu
    IDF = mybir.ActivationFunctionType.Identity
    SQRTF = mybir.ActivationFunctionType.Sqrt

    with tile.TileContext(nc) as tc:
        with (
            tc.tile_pool(name="persist", bufs=1) as pp,
            tc.tile_pool(name="bigA", bufs=1) as bigA,     # xb_sb then W1
            tc.tile_pool(name="bigB", bufs=1) as bigB,     # xT then W2
            tc.tile_pool(name="stage", bufs=2) as stp,     # heat staging, then h1T
            tc.tile_pool(name="work", bufs=4) as wk,
            tc.tile_pool(name="work2", bufs=2) as wk2,       # P tiles etc.
            tc.tile_pool(name="small", bufs=1) as sm,
            tc.tile_pool(name="ps_score", bufs=2, space="PSUM") as psS,
            tc.tile_pool(name="ps_tr", bufs=2, space="PSUM") as psT,
            tc.tile_pool(name="ps_wv", bufs=2, space="PSUM") as psW,
            tc.tile_pool(name="ps_misc", bufs=2, space="PSUM") as psM,
        ):
            # ============ load phase ============
            xb_sb = bigA.tile([128, TC, D], BF16, tag="bigA")
            nc.sync.dma_start(out=xb_sb, in_=xb[:].rearrange("(a p) d -> p a d", p=128))
            xq_sb = pp.tile([128, 4, D], F32)
            nc.sync.dma_start(out=xq_sb, in_=xq[:].rearrange("(a p) d -> p a d", p=128))
            mask_sb = pp.tile([128, TC, R], BF16)
            nc.sync.dma_start(out=mask_sb, in_=maskin[:].rearrange("(a p) s -> p a s", p=128))
            ident = sm.tile([128, 128], BF16)
            nc.sync.dma_start(out=ident, in_=identin[:])

            Wq_sb = pp.tile([128, 4, D], BF16)
            Wk_sb = pp.tile([128, 4, D], BF16)
            Wv_sb = pp.tile([128, 4, D], BF16)
            Wo_sb = pp.tile([128, 4, D], BF16)
            for wsb, wdr in ((Wq_sb, Wq), (Wk_sb, Wk), (Wv_sb, Wv), (Wo_sb, Wo)):
                nc.sync.dma_start(out=wsb, in_=wdr[:].rearrange("(c p) n -> p c n", p=128))

            # bias columns [128, nchunk] (per-partition use)
            bqc = sm.tile([128, 4], F32)
            nc.sync.dma_start(out=bqc, in_=bq[:].rearrange("(c p) -> p c", p=128))
            bkc = sm.tile([128, 4], F32)
            nc.sync.dma_start(out=bkc, in_=bk[:].rearrange("(c p) -> p c", p=128))
            b1c = sm.tile([128, 16], F32)
            nc.sync.dma_start(out=b1c, in_=b1[:].rearrange("(c p) -> p c", p=128))
            # broadcast rows [128, D] (per-free use)
            def brow(vec):
                t = sm.tile([128, D], F32, tag=f"brow_{vec.name}")
                ap = bass.AP(tensor=vec[:].tensor, offset=0, ap=[[0, 128], [1, D]])
                nc.gpsimd.dma_start(out=t, in_=ap)
                return t
            bv_t, bo_t, b2_t = brow(bv), brow(bo), brow(b2)
            g1_t, bt1_t, g2_t, bt2_t = brow(g1), brow(bt1), brow(g2), brow(bt2)
            epsc = sm.tile([128, 1], F32)
            nc.vector.memset(epsc, LN_EPS)

            # ---- wait-fanin preamble ----
            # walrus in this toolchain encodes at most ONE sync-wait per
            # instruction, so each engine observes every input-DMA semaphore
            # lane once, via cheap touch ops writing disjoint scratch.
            dve_scr = sm.tile([1, 16], F32)
            for i, t in enumerate((xq_sb, mask_sb, bqc, bkc, bv_t, bo_t, b2_t,
                                   g1_t, bt1_t, g2_t, bt2_t)):
                srcap = t[0:1, 0:1] if len(t.shape) == 2 else t[0:1, 0, 0:1]
                nc.vector.tensor_copy(dve_scr[:, i:i + 1], srcap)
            for t in (xb_sb, Wq_sb, Wk_sb, Wv_sb, Wo_sb):
                nc.tensor.ldweights(weights=t[0:1, 0, 0:1])
            nc.tensor.ldweights(weights=ident[0:1, 0:1])
            act_scr = sm.tile([1, 4], F32)
            nc.scalar.copy(act_scr[:, 0:1], b1c[0:1, 0:1])
            gps_scr = sm.tile([1, 4], F32)
            nc.gpsimd.tensor_copy(gps_scr[:, 0:1], mask_sb[0:1, 0, 0:1])

            # ============ transpose x -> xT [d, t] ============
            xT = bigB.tile([128, 4, S], BF16, tag="bigB")
            for dblk in range(4):
                for tg in range(4):
                    pst = psT.tile([128, 512], BF16, tag="tr")
                    for i in range(4):
                        tcc = tg * 4 + i
                        nc.tensor.transpose(
                            pst[:, i * 128:(i + 1) * 128],
                            xb_sb[:, tcc, dblk * 128:(dblk + 1) * 128], ident)
                    nc.vector.tensor_copy(xT[:, dblk, tg * 512:(tg + 1) * 512], pst)

            # ============ projections ============
            QT = pp.tile([128, 4, R], BF16)   # [hq, s]  (s = rolled rows 0..511)
            for mc in range(4):
                ps = psM.tile([128, 512], F32, tag="misc")
                for kc in range(4):
                    nc.tensor.matmul(ps, Wq_sb[:, kc, mc * 128:(mc + 1) * 128],
                                     xT[:, kc, 0:R], start=(kc == 0), stop=(kc == 3))
                nc.vector.tensor_scalar_add(QT[:, mc, :], ps, bqc[:, mc:mc + 1])
            KT = pp.tile([128, 4, S], BF16)   # [hq, t]
            for mc in range(4):
                for ng in range(4):
                    ps = psM.tile([128, 512], F32, tag="misc")
                    for kc in range(4):
                        nc.tensor.matmul(ps, Wk_sb[:, kc, mc * 128:(mc + 1) * 128],
                                         xT[:, kc, ng * 512:(ng + 1) * 512],
                                         start=(kc == 0), stop=(kc == 3))
                    nc.vector.tensor_scalar_add(KT[:, mc, ng * 512:(ng + 1) * 512],
                                                ps, bkc[:, mc:mc + 1])
            # V with ones column appended per head: layout [t, 16, 8*65]
            V_sb = pp.tile([128, TC, H * (VD + 1)], BF16)
            for tcc in range(TC):
                ps = psM.tile([128, 512], F32, tag="misc")
                for kc in range(4):
                    nc.tensor.matmul(ps, xT[:, kc, tcc * 128:(tcc + 1) * 128],
                                     Wv_sb[:, kc, :], start=(kc == 0), stop=(kc == 3))
                vout = V_sb[:, tcc, :].rearrange("p (h v) -> p h v", v=VD + 1)[:, :, 0:VD]
                nc.vector.tensor_add(vout, ps.rearrange("p (h v) -> p h v", h=H),
                                     bv_t.rearrange("p (h v) -> p h v", h=H))
            ones_ap = V_sb[:, :, :].rearrange("p a (h v) -> p a h v", v=VD + 1)[:, :, :, VD:VD + 1]
            nc.vector.memset(ones_ap, 1.0)

            # FFN weights: DMA after projections (slots shared with xb_sb/xT)
            W1_sb = bigA.tile([128, 4, DH], BF16, tag="bigA")
            nc.sync.dma_start(out=W1_sb, in_=W1[:].rearrange("(c p) n -> p c n", p=128))
            W2_sb = bigB.tile([128, 16, D], BF16, tag="bigB")
            nc.sync.dma_start(out=W2_sb, in_=W2[:].rearrange("(c p) n -> p c n", p=128))
            nc.tensor.ldweights(weights=W1_sb[0:1, 0, 0:1])
            nc.tensor.ldweights(weights=W2_sb[0:1, 0, 0:1])

            # ============ attention ============
            wvT = pp.tile([128, 4, R], BF16)  # normalized (P@V)^T stacked heads [hv, s]
            for h in range(H):
                pb, ch = (h % 2) * 64, h // 2
                wv_ps = psW.tile([65, 512], F32, tag="wv")
                stg = None
                for tcc in range(TC):
                    if tcc % 8 == 0:
                        stg = stp.tile([128, 8, R], F32, tag="stage")
                        # claim: absorb the WAR-on-heat-DMA wait in a 1-elem op
                        nc.scalar.copy(stg[0:1, 0, 0:1], b1c[0:1, 0:1])
                    ps = psS.tile([128, 512], F32, tag="score")
                    nc.tensor.matmul(ps, KT[pb:pb + 64, ch, tcc * 128:(tcc + 1) * 128],
                                     QT[pb:pb + 64, ch, :], start=True, stop=True)
                    # raw scores -> staging (ACT), exp -> P (ACT)
                    nc.scalar.copy(stg[:, tcc % 8, :], ps)
                    P = wk.tile([128, R], BF16, tag="P")
                    nc.scalar.activation(out=P, in_=ps, func=EXPF)
                    # causal mask multiply (split DVE / GPSIMD)
                    eng = nc.vector if tcc % 2 == 0 else nc.gpsimd
                    eng.tensor_mul(P, P, mask_sb[:, tcc, :])
                    # accumulate (P@V)^T with ones row
                    nc.tensor.matmul(wv_ps, V_sb[:, tcc, h * 65:(h + 1) * 65], P,
                                     start=(tcc == 0), stop=(tcc == TC - 1))
                    if tcc % 8 == 7:
                        half = tcc // 8
                        dst = heatT[h, half * 1024:(half + 1) * 1024, :]
                        nc.sync.dma_start(out=dst.rearrange("(a p) s -> p a s", p=128),
                                          in_=stg)
                # normalize by softmax sums (row 64) and store transposed-stacked
                rrow = wk2.tile([1, 512], F32, tag="rrow")
                nc.vector.reciprocal(rrow, wv_ps[64:65, :])
                rt = wk2.tile([64, 512], F32, tag="rt")
                nc.gpsimd.partition_broadcast(rt, rrow)
                nc.vector.tensor_mul(wvT[pb:pb + 64, ch, :], wv_ps[0:64, :], rt)

            # ============ attention out + LN1 ============
            # xq_sb += bo (residual + bias prefold)
            for sc in range(4):
                nc.vector.tensor_add(xq_sb[:, sc, :], xq_sb[:, sc, :], bo_t)
            x1 = pp.tile([128, 4, D], F32)
            x1bf = pp.tile([128, 4, D], BF16)

            def layernorm(src_ps, resid, g_t, b_t, out_f32, out_bf, sc):
                pre = wk2.tile([128, D], F32, tag="lnpre")
                nc.vector.tensor_add(pre, src_ps, resid)
                st6 = wk2.tile([128, 6], F32, tag="st6")
                nc.vector.bn_stats(st6, pre)
                mv = wk2.tile([128, 2], F32, tag="mv")
                nc.vector.bn_aggr(mv, st6)
                rstd = wk2.tile([128, 1], F32, tag="rstd")
                nc.scalar.activation(out=rstd, in_=mv[:, 1:2], func=SQRTF, bias=epsc, scale=1.0)
                nc.vector.reciprocal(rstd, rstd)
                negmr = wk2.tile([128, 1], F32, tag="negmr")
                nc.vector.tensor_scalar_mul(negmr, mv[:, 0:1], -1.0)
                nc.vector.tensor_mul(negmr, negmr, rstd)
                xn = wk2.tile([128, D], F32, tag="xn")
                nc.scalar.activation(out=xn, in_=pre, func=IDF, bias=negmr, scale=rstd)
                gx = wk2.tile([128, D], F32, tag="gx")
                nc.vector.tensor_mul(gx, xn, g_t)
                nc.vector.tensor_add(out_f32[:, sc, :], gx, b_t)
                if out_bf is not None:
                    nc.vector.tensor_copy(out_bf[:, sc, :], out_f32[:, sc, :])

            for sc in range(4):
                ps = psM.tile([128, 512], F32, tag="misc")
                for kc in range(4):
                    nc.tensor.matmul(ps, wvT[:, kc, sc * 128:(sc + 1) * 128],
                                     Wo_sb[:, kc, :], start=(kc == 0), stop=(kc == 3))
                layernorm(ps, xq_sb[:, sc, :], g1_t, bt1_t, x1, x1bf, sc)

            # ============ FFN ============
            x1T = pp.tile([128, 4, R], BF16)
            for dblk in range(4):
                pst = psT.tile([128, 512], BF16, tag="tr")
                for sc in range(4):
                    nc.tensor.transpose(pst[:, sc * 128:(sc + 1) * 128],
                                        x1bf[:, sc, dblk * 128:(dblk + 1) * 128], ident)
                nc.vector.tensor_copy(x1T[:, dblk, :], pst)
            h1T = stp.tile([128, 16, R], BF16, tag="stage")
            nc.scalar.copy(h1T[0:1, 0, 0:1], b1c[0:1, 0:1])
            for mc in range(16):
                ps = psM.tile([128, 512], F32, tag="misc")
                for kc in range(4):
                    nc.tensor.matmul(ps, W1_sb[:, kc, mc * 128:(mc + 1) * 128],
                                     x1T[:, kc, :], start=(kc == 0), stop=(kc == 3))
                nc.scalar.activation(out=h1T[:, mc, :], in_=ps, func=RELUF,
                                     bias=b1c[:, mc:mc + 1], scale=1.0)
            # x1 += b2 (prefold residual+bias for LN2)
            for sc in range(4):
                nc.vector.tensor_add(x1[:, sc, :], x1[:, sc, :], b2_t)
            x2_sb = pp.tile([128, 4, D], F32)
            for sc in range(4):
                ps = psM.tile([128, 512], F32, tag="misc")
                for kc in range(16):
                    nc.tensor.matmul(ps, h1T[:, kc, sc * 128:(sc + 1) * 128],
                                     W2_sb[:, kc, :], start=(kc == 0), stop=(kc == 15))
                layernorm(ps, x1[:, sc, :], g2_t, bt2_t, x2_sb, None, sc)
            nc.sync.dma_start(out=x2out[:].rearrange("(a p) d -> p a d", p=128), in_=x2_sb)
    nc.compile()
    return nc


def _get_nc():
    if "nc" not in _NC_CACHE:
        _NC_CACHE["nc"] = build_nc()
    return _NC_CACHE["nc"]


def make_in_maps(inputs):
    x = np.asarray(inputs["x"], dtype=np.float32)
    B = x.shape[0]
    shared = {}
    for k in ("Wq", "Wk", "Wv", "Wo", "W1", "W2"):
        shared[k] = np.asarray(inputs[k], dtype=np.float32).astype(BF)
    for k_src, k_dst in (("bq", "bq"), ("bk", "bk"), ("bv", "bv"), ("bo", "bo"),
                         ("b1", "b1"), ("b2", "b2"), ("ln1_g", "g1"), ("ln1_b", "bt1"),
                         ("ln2_g", "g2"), ("ln2_b", "bt2")):
        shared[k_dst] = np.ascontiguousarray(np.asarray(inputs[k_src], dtype=np.float32))
    shared["identin"] = np.eye(128, dtype=np.float32).astype(BF)
    in_maps = []
    jj = np.arange(R)
    tg = np.arange(S)
    for c in range(N_CORES):
        b, gs = c // 4, (c % 4) * R
        xb_rolled = np.roll(x[b], -gs, axis=0).astype(BF)
        xq_c = np.ascontiguousarray(x[b, gs:gs + R])
        m = (tg[:, None] <= (gs + jj)[None, :]).astype(np.float32)
        m_rolled = np.roll(m, -gs, axis=0).astype(BF)
        in_maps.append(dict(shared, xb=xb_rolled, xq=xq_c, maskin=m_rolled))
    return in_maps


def assemble(results):
    x2 = np.empty((2, S, D), np.float32)
    heatmap = np.empty((H * 2, S, S), np.float32)
    for c in range(N_CORES):
        b, gs = c // 4, (c % 4) * R
        r = results[c]
        x2[b, gs:gs + R] = r["x2out"]
        hT = np.roll(r["heatT"], gs, axis=1)          # [H, t, s] unrolled
        heatmap[b::2, gs:gs + R, :] = hT.transpose(0, 2, 1)
    return x2, heatmap


def kernel(**inputs):
    nc = _get_nc()
    in_maps = make_in_maps(inputs)
    res = run_bass_kernel_spmd(nc, in_maps, core_ids=list(range(N_CORES)))
    return assemble(res.results)
